# revision 1
# baseline (speedup 1.0000x reference)
"""Bass/Trainium2 kernel for nn_Attention_13615046328582.

Causal multi-head attention with RoPE, B=4 S=2048 E=2048 H=16 D=128, fp32.
Sharding: 4-way batch DP x 2-way head TP across 8 NeuronCores.
Each core: 1 batch, 8 heads. Host sums the TP pair partials + bo.

Per-core plan (all matmuls fp32r = full-rate 1 cyc/row at N>=512):
  P1  V/Q/K projections, weights streamed in 4MB halves through a 3-slot
      pool (next pass prefetches under current pass). Q/K evacuate through
      ACT (bias add) then RoPE = pair-swap permutation matmul + 3 DVE ops
      against host-built [d,t] cos / signed-sin tables; spilled to DRAM as
      qT/kT [d,t] per head and v [t,f].
  P2  per (head, 512-query block): S^T logits [k,q] (K^T stationary),
      exp on ACT straight out of PSUM (scale folded), diagonal blocks
      masked by a bf16 0/1 mask multiply, row sums via ones-vector matmul,
      V-bias folded in exactly as a rank-1 (bv x sums) matmul, softmax
      normalization via ones-broadcast matmul + DVE multiply.
  P3  out-projection [t,e] accumulating 8 head blocks per PSUM tile; Wo
      prefetched 1MB/head during P2.
Measured (NTFF via axon nrt-profile): 945.8us/core, max rel err 2.8e-4.
"""

import numpy as np
import ml_dtypes
from contextlib import ExitStack

import concourse.bass as bass
import concourse.tile as tile
from concourse import bacc, mybir
from concourse.bass_utils import run_bass_kernel_spmd

B, S, E, H = 4, 2048, 2048, 16
D = E // H            # 128 head dim
HL = 8                # heads per core
FL = HL * D           # 1024 local features
N_CORES = 8
ROPE_BASE = 10000.0
SCALE = float(D) ** -0.5
F32 = mybir.dt.float32
F32R = mybir.dt.float32r
BF16 = mybir.dt.bfloat16
Act = mybir.ActivationFunctionType

NE = E // 128         # 16 e-tiles
NT = S // 512         # 4 t-blocks of 512
NTT = S // 128        # 16 t-tiles of 128
NFO = FL // 128       # 8 f-tiles (= local heads)


def _r(ap):
    return ap


def build_nc():
    nc = bacc.Bacc(
        "TRN2", target_bir_lowering=False, debug=False, num_devices=N_CORES
    )
    xT = nc.dram_tensor("xT", [E, S], F32R, kind="ExternalInput").ap()
    wq = nc.dram_tensor("wq", [E, FL], F32R, kind="ExternalInput").ap()
    wk = nc.dram_tensor("wk", [E, FL], F32R, kind="ExternalInput").ap()
    wv = nc.dram_tensor("wv", [E, FL], F32R, kind="ExternalInput").ap()
    wo = nc.dram_tensor("wo", [FL, E], F32R, kind="ExternalInput").ap()
    bq = nc.dram_tensor("bq", [128, NFO], F32, kind="ExternalInput").ap()
    bk = nc.dram_tensor("bk", [128, NFO], F32, kind="ExternalInput").ap()
    bv = nc.dram_tensor("bv", [1, FL], F32R, kind="ExternalInput").ap()
    cosT = nc.dram_tensor("cosT", [128, S], F32, kind="ExternalInput").ap()
    sinST = nc.dram_tensor("sinST", [128, S], F32, kind="ExternalInput").ap()
    pswap = nc.dram_tensor("pswap", [128, 128], F32R, kind="ExternalInput").ap()
    ones_col_d = nc.dram_tensor("ones_col", [128, 1], F32R, kind="ExternalInput").ap()
    ones_row_d = nc.dram_tensor("ones_row", [1, 128], F32R, kind="ExternalInput").ap()
    masks = nc.dram_tensor("masks", [128, 4, 512], BF16, kind="ExternalInput").ap()
    out = nc.dram_tensor("out", [S, E], F32, kind="ExternalOutput").ap()

    with tile.TileContext(nc) as tc, ExitStack() as top:
        dram = top.enter_context(tc.tile_pool(name="dram", bufs=1, space="DRAM"))
        qT_d = [dram.tile([128, S], F32R, tag=f"qT{i}", name=f"qT_d{i}") for i in range(NFO)]
        kT_d = [dram.tile([128, S], F32R, tag=f"kT{i}", name=f"kT_d{i}") for i in range(NFO)]
        v_d = dram.tile([S, FL], F32R)
        aoT_d = dram.tile([FL, S], F32R)

        cpool = top.enter_context(tc.tile_pool(name="const", bufs=1))
        ones_col = cpool.tile([128, 1], F32R, tag="ones_col")
        nc.sync.dma_start(ones_col[:], ones_col_d[:])
        ones_row = cpool.tile([1, 128], F32R, tag="ones_row")
        nc.sync.dma_start(ones_row[:], ones_row_d[:])
        zb = cpool.tile([128, 1], F32, tag="zb")
        nc.gpsimd.memset(zb[:], 0.0)
        pswap_sb = cpool.tile([128, 128], F32R, tag="pswap")
        nc.sync.dma_start(pswap_sb[:], pswap[:])
        bq_sb = cpool.tile([128, NFO], F32, tag="bq")
        nc.sync.dma_start(bq_sb[:], bq[:])
        bk_sb = cpool.tile([128, NFO], F32, tag="bk")
        nc.sync.dma_start(bk_sb[:], bk[:])
        bv_sb = cpool.tile([1, FL], F32R, tag="bv")
        nc.sync.dma_start(bv_sb[:], bv[:])

        xT_r = xT.rearrange("(eo p) t -> p eo t", p=128)

        # ---- Phase 1: Q/K/V projections. Weights stream in 4MB halves
        # through one shared 3-slot pool so the next pass's weights prefetch
        # under the current pass's matmuls (removes the inter-pass PE gaps).
        with ExitStack() as ph:
            c1 = ph.enter_context(tc.tile_pool(name="c1", bufs=1))
            wp = ph.enter_context(tc.tile_pool(name="wqk", bufs=3))
            xp = ph.enter_context(tc.tile_pool(name="xs", bufs=2))
            ps = ph.enter_context(tc.tile_pool(name="ps1", bufs=4, space="PSUM"))
            ps2 = ph.enter_context(tc.tile_pool(name="ps1b", bufs=2, space="PSUM"))
            st = ph.enter_context(tc.tile_pool(name="st1", bufs=3))

            def load_w_half(w_in, half):
                t = wp.tile([128, NE, 512], F32R, tag="w")
                nc.sync.dma_start(
                    t[:],
                    w_in.rearrange("(eo p) f -> p eo f", p=128)[
                        :, :, half * 512 : (half + 1) * 512
                    ],
                )
                return t

            # V pass first: its outputs are P2's widest dependency (every
            # v_h column read touches all 32 V stores), so give it the
            # longest runway before phase 2 starts.
            wv_half = [load_w_half(wv, 0)]
            x_first = xp.tile([128, NE, 512], F32R, tag="x")
            nc.sync.dma_start(x_first[:], xT_r[:, :, 0:512])
            wv_half.append(load_w_half(wv, 1))
            for tb in range(NT):
                if tb == 0:
                    x_sb = x_first
                else:
                    x_sb = xp.tile([128, NE, 512], F32R, tag="x")
                    nc.sync.dma_start(
                        x_sb[:], xT_r[:, :, tb * 512 : (tb + 1) * 512]
                    )
                for ttl in range(4):
                    tt = tb * 4 + ttl
                    for fo2 in range(2):
                        acc = ps.tile([128, 512], F32)
                        for eo in range(NE):
                            nc.tensor.matmul(
                                acc[:],
                                x_sb[:, eo, ttl * 128 : (ttl + 1) * 128],
                                wv_half[fo2][:, eo, :],
                                start=(eo == 0),
                                stop=(eo == NE - 1),
                            )
                        vt = st.tile([128, 512], F32R, tag="raw")
                        nc.scalar.mul(vt[:], acc[:], 1.0)
                        nc.sync.dma_start(
                            v_d[tt * 128 : (tt + 1) * 128,
                                fo2 * 512 : (fo2 + 1) * 512],
                            vt[:],
                        )

            # cos/sin are only needed from the Q pass on; issued after the
            # V-pass loads so they don't delay the first matmul.
            cos_sb = c1.tile([128, S], F32, tag="cos")
            nc.sync.dma_start(cos_sb[:], cosT[:])
            sin_sb = c1.tile([128, S], F32, tag="sin")
            nc.sync.dma_start(sin_sb[:], sinST[:])

            # Q and K passes: out qT/kT [f, t] + bias + RoPE
            for w_in, b_sb, dst in ((wq, bq_sb, qT_d), (wk, bk_sb, kT_d)):
                w_half = [load_w_half(w_in, 0), load_w_half(w_in, 1)]
                for tb in range(NT):
                    x_sb = xp.tile([128, NE, 512], F32R, tag="x")
                    nc.sync.dma_start(
                        x_sb[:], xT_r[:, :, tb * 512 : (tb + 1) * 512]
                    )
                    for fo in range(NFO):
                        w_sb = w_half[fo // 4]
                        fl = (fo % 4) * 128
                        acc = ps.tile([128, 512], F32)
                        for eo in range(NE):
                            nc.tensor.matmul(
                                acc[:],
                                w_sb[:, eo, fl : fl + 128],
                                x_sb[:, eo, :],
                                start=(eo == 0),
                                stop=(eo == NE - 1),
                            )
                        raw = st.tile([128, 512], F32R, tag="raw")
                        nc.scalar.activation(
                            raw[:], acc[:], Act.Identity,
                            bias=b_sb[:, fo : fo + 1],
                        )
                        swp = ps2.tile([128, 512], F32)
                        nc.tensor.matmul(
                            swp[:], pswap_sb[:], raw[:],
                            start=True, stop=True,
                        )
                        t1 = st.tile([128, 512], F32R, tag="t1")
                        nc.vector.tensor_mul(
                            t1[:], raw[:], cos_sb[:, tb * 512 : (tb + 1) * 512]
                        )
                        t2 = st.tile([128, 512], F32R, tag="t2")
                        nc.vector.tensor_mul(
                            t2[:], swp[:], sin_sb[:, tb * 512 : (tb + 1) * 512]
                        )
                        fin = st.tile([128, 512], F32R, tag="fin")
                        nc.vector.tensor_add(fin[:], t1[:], t2[:])
                        nc.sync.dma_start(
                            dst[fo][:, tb * 512 : (tb + 1) * 512], fin[:]
                        )

        # Wo prefetch: pool opened outside P2/P3 scopes so the 8MB load
        # streams in under phase-2 compute instead of stalling phase 3.
        wp3 = top.enter_context(tc.tile_pool(name="wo", bufs=1))
        wo_sb = wp3.tile([128, NFO, E], F32R)
        wo_r = wo.rearrange("(fo p) e -> p fo e", p=128)

        # ---- Phase 2: attention per head -> aoT_d [f, t]
        with ExitStack() as ph:
            hp = ph.enter_context(tc.tile_pool(name="heads", bufs=2))
            ep = ph.enter_context(tc.tile_pool(name="expS", bufs=2))
            psS = ph.enter_context(tc.tile_pool(name="psS", bufs=3, space="PSUM"))
            psSum = ph.enter_context(
                tc.tile_pool(name="psSum", bufs=2, space="PSUM")
            )
            psO = ph.enter_context(tc.tile_pool(name="psO", bufs=2, space="PSUM"))
            psB = ph.enter_context(tc.tile_pool(name="psB", bufs=1, space="PSUM"))
            sm = ph.enter_context(tc.tile_pool(name="sm2", bufs=2))
            st = ph.enter_context(tc.tile_pool(name="st2", bufs=3))
            c2 = ph.enter_context(tc.tile_pool(name="c2", bufs=1))
            mask_sb = c2.tile([128, 4, 512], BF16, tag="mask")
            nc.sync.dma_start(mask_sb[:], masks[:])
            for h in range(HL):
                qT_h = hp.tile([128, S], F32R, tag="qh")
                nc.sync.dma_start(qT_h[:], qT_d[h][:])
                kT_h = hp.tile([128, S], F32R, tag="kh")
                nc.sync.dma_start(kT_h[:], kT_d[h][:])
                v_h = hp.tile([128, NTT, 128], F32R, tag="vh")
                nc.sync.dma_start(
                    v_h[:],
                    v_d[:, h * 128 : (h + 1) * 128].rearrange(
                        "(j p) d -> p j d", p=128
                    ),
                )
                # one 1MB slice of Wo per head: full tile ready before P3
                nc.scalar.dma_start(wo_sb[:, h, :], wo_r[:, h, :])
                for b in range(NT):
                    nk = 4 * b + 4
                    eS = ep.tile([128, NTT, 512], F32R)
                    for j in range(nk):
                        ls = psS.tile([128, 512], F32)
                        nc.tensor.matmul(
                            ls[:],
                            _r(kT_h[:, j * 128 : (j + 1) * 128]),
                            _r(qT_h[:, b * 512 : (b + 1) * 512]),
                            start=True, stop=True,
                        )
                        nc.scalar.activation(
                            eS[:, j, :], ls[:], Act.Exp, bias=zb[:, 0:1],
                            scale=SCALE,
                        )
                        if j >= 4 * b:
                            nc.vector.tensor_mul(
                                eS[:, j, :], eS[:, j, :],
                                mask_sb[:, j - 4 * b, :],
                            )
                    ssum = psSum.tile([1, 512], F32)
                    for j in range(nk):
                        nc.tensor.matmul(
                            ssum[:], _r(ones_col[:]), _r(eS[:, j, :]),
                            start=(j == 0), stop=(j == nk - 1),
                        )
                    sums_sb = sm.tile([1, 512], F32R, tag="sums")
                    nc.vector.tensor_copy(sums_sb[:], ssum[:])
                    recip_sb = sm.tile([1, 512], F32R, tag="recip")
                    with nc.allow_low_precision(reason="f32r matmul operand"):
                        nc.vector.reciprocal(recip_sb[:], ssum[:])
                    pso = psO.tile([128, 512], F32)
                    for j in range(nk):
                        nc.tensor.matmul(
                            pso[:], _r(v_h[:, j, :]), _r(eS[:, j, :]),
                            start=(j == 0), stop=False,
                        )
                    nc.tensor.matmul(
                        pso[:],
                        _r(bv_sb[0:1, h * 128 : (h + 1) * 128]),
                        _r(sums_sb[:]),
                        start=False, stop=True,
                    )
                    psb = psB.tile([128, 512], F32)
                    nc.tensor.matmul(
                        psb[:], _r(ones_row[:]), _r(recip_sb[:]),
                        start=True, stop=True,
                    )
                    bcast = st.tile([128, 512], F32, tag="bcast")
                    nc.vector.tensor_copy(bcast[:], psb[:])
                    attn = st.tile([128, 512], F32R, tag="attn")
                    nc.vector.tensor_mul(attn[:], pso[:], bcast[:])
                    nc.sync.dma_start(
                        aoT_d[h * 128 : (h + 1) * 128,
                              b * 512 : (b + 1) * 512],
                        attn[:],
                    )

        # ---- Phase 3: out proj -> out [t, e] (bo added on host)
        with ExitStack() as ph:
            ap_ = ph.enter_context(tc.tile_pool(name="ao", bufs=2))
            ps = ph.enter_context(tc.tile_pool(name="ps3", bufs=4, space="PSUM"))
            st = ph.enter_context(tc.tile_pool(name="st3", bufs=3))
            for tt in range(NTT):
                ao_sb = ap_.tile([128, NFO, 128], F32R)
                nc.sync.dma_start(
                    ao_sb[:],
                    aoT_d[:, tt * 128 : (tt + 1) * 128].rearrange(
                        "(fo p) t -> p fo t", p=128
                    ),
                )
                for eb in range(NT):
                    acc = ps.tile([128, 512], F32)
                    for fo in range(NFO):
                        nc.tensor.matmul(
                            acc[:],
                            _r(ao_sb[:, fo, :]),
                            _r(wo_sb[:, fo, eb * 512 : (eb + 1) * 512]),
                            start=(fo == 0),
                            stop=(fo == NFO - 1),
                        )
                    osb = st.tile([128, 512], F32)
                    nc.scalar.mul(osb[:], acc[:], 1.0)
                    nc.sync.dma_start(
                        out[tt * 128 : (tt + 1) * 128,
                            eb * 512 : (eb + 1) * 512],
                        osb[:],
                    )

    nc.compile()
    return nc


def _host_inputs(x, Wq, bq, Wk, bk, Wv, bv, Wo, bo):
    # RoPE tables in [d, t] layout, shared across heads.
    i = np.arange(0, D, 2, dtype=np.float64)
    invf = ROPE_BASE ** (-i / D)                      # (64,)
    pos = np.arange(S, dtype=np.float64)
    ang = pos[None, :] * invf[:, None]                # (64, S)
    cosT = np.empty((128, S), np.float32)
    sinST = np.empty((128, S), np.float32)
    cosT[0::2] = np.cos(ang)
    cosT[1::2] = np.cos(ang)
    sinST[0::2] = -np.sin(ang)
    sinST[1::2] = np.sin(ang)
    pswap = np.zeros((128, 128), np.float32)
    idx = np.arange(128)
    pswap[idx, idx ^ 1] = 1.0
    # Causal masks for the 4 diagonal-straddling alignments.
    ki = np.arange(128)[:, None, None]
    m = np.arange(4)[None, :, None]
    qi = np.arange(512)[None, None, :]
    masks = (qi >= ki + 128 * m).astype(np.float32)   # (128, 4, 512)

    in_maps = []
    for c in range(N_CORES):
        bi, g = c % B, c // B
        sl = slice(g * FL, (g + 1) * FL)
        in_maps.append({
            "xT": np.ascontiguousarray(x[bi].T),
            "wq": np.ascontiguousarray(Wq[sl, :].T),
            "wk": np.ascontiguousarray(Wk[sl, :].T),
            "wv": np.ascontiguousarray(Wv[sl, :].T),
            "wo": np.ascontiguousarray(Wo[:, sl].T),
            "bq": np.ascontiguousarray(bq[sl].reshape(NFO, 128).T),
            "bk": np.ascontiguousarray(bk[sl].reshape(NFO, 128).T),
            "bv": np.ascontiguousarray(bv[sl].reshape(1, FL)),
            "cosT": cosT,
            "sinST": sinST,
            "pswap": pswap,
            "masks": masks.astype(ml_dtypes.bfloat16),
            "ones_col": np.ones((128, 1), np.float32),
            "ones_row": np.ones((1, 128), np.float32),
        })
    return in_maps


_NC_CACHE = {}


def _install_ntff_hook():
    """Recreate the missing antenv.axon_hooks module so trace=True works.

    The axon PJRT .so exports axon_start/stop_nrt_profile; only the thin
    python wrapper is absent from this image. Mirrors
    trn_agent_boot._ntff_profile_via_ctypes.
    """
    import sys, types, ctypes, contextlib

    if "antenv.axon_hooks" in sys.modules:
        return
    so_path = "/opt/axon/libaxon_pjrt.so"
    lib = ctypes.CDLL(so_path)
    lib.axon_start_nrt_profile.argtypes = [
        ctypes.POINTER(ctypes.c_int64), ctypes.c_size_t,
    ]
    lib.axon_start_nrt_profile.restype = ctypes.c_int64
    lib.axon_stop_nrt_profile.argtypes = [ctypes.c_char_p]
    lib.axon_stop_nrt_profile.restype = ctypes.c_int64

    @contextlib.contextmanager
    def _hook(output_dir, device_ids):
        import jax
        jax.devices()
        if device_ids:
            ids = (ctypes.c_int64 * len(device_ids))(*device_ids)
            rc = lib.axon_start_nrt_profile(ids, len(device_ids))
        else:
            rc = lib.axon_start_nrt_profile(None, 0)
        if rc != 0:
            raise RuntimeError(f"axon_start_nrt_profile rc={rc}")
        try:
            yield
        finally:
            n = lib.axon_stop_nrt_profile(str(output_dir).encode())
            print(f"ntff profile: {n} file(s) -> {output_dir}", flush=True)

    mod = types.ModuleType("antenv.axon_hooks")
    mod.get_axon_ntff_profile_hook = lambda: _hook
    mod.set_axon_ntff_profile_hook = lambda h: None
    sys.modules["antenv.axon_hooks"] = mod
    # S3 artifact upload has no creds here; neutralize it.
    import concourse.bass_utils as bu
    bu.upload_artifacts = lambda tmpdir: f"file://{tmpdir}"


def run(x, Wq, bq, Wk, bk, Wv, bv, Wo, bo, trace=False):
    if trace:
        try:
            _install_ntff_hook()
        except Exception as e:
            print(f"ntff hook install failed ({e}); tracing may degrade")
    if "nc" not in _NC_CACHE:
        _NC_CACHE["nc"] = build_nc()
    nc = _NC_CACHE["nc"]
    in_maps = _host_inputs(x, Wq, bq, Wk, bk, Wv, bv, Wo, bo)
    res = run_bass_kernel_spmd(
        nc, in_maps, core_ids=list(range(N_CORES)), trace=trace
    )
    outs = [res.results[c]["out"] for c in range(N_CORES)]
    full = np.empty((B, S, E), np.float32)
    for bi in range(B):
        full[bi] = outs[bi] + outs[bi + B] + bo[None, :]
    return full, res


def kernel(**inputs):
    full, _ = run(**inputs)
    return full



# revision 8
# speedup vs baseline: 1.2563x; 1.2563x over previous
"""Bass/Trainium2 kernel for nn_Attention_13615046328582.

Causal multi-head attention with RoPE, B=4 S=2048 E=2048 H=16 D=128, fp32 io.
Sharding: 4-way batch DP x 2-way head TP across 8 NeuronCores.
Each core: 1 batch, 8 heads. Host sums the TP pair partials + bo.

All matmuls run bf16 (same PE rate as fp32r but FWL weight loads hide
LDWEIGHTS; halves DMA/SBUF). Numerics validated by sim: metric ~4e-3 vs
the 2e-2 gate.

Per-core plan:
  P1  V/Q/K projections; weights stream in 4MB->2MB bf16 halves through a
      3-slot pool. Q/K features are host-permuted to [evens|odds] per head
      so RoPE's pair swap becomes a half-tile partition swap done entirely
      on DVE (2 muls + 2 half adds vs a PE permutation matmul). Q/K land
      resident in SBUF [128, h, S]; V spills to DRAM [S, FL] bf16.
  P2  per (head, 512-q block), fine-grained causal: diagonal-straddling
      k-tiles only stream their valid q-suffix; true-diagonal 128x128
      masked by a bf16 triangle multiply. exp on ACT batched 2 PSUM banks
      per op; row sums via ones-vector matmuls; 1/sums via DVE
      reciprocal_approx_fast; softmax normalize + ao store deferred one
      block so PE never waits; ao kept resident in SBUF bf16.
  P3  out-projection [t,e] straight from resident ao/wo (wo prefetched
      during P2); f32 evac + store; bo added on host.
"""

import numpy as np
import ml_dtypes
from contextlib import ExitStack

import concourse.bass as bass
import concourse.tile as tile
from concourse import bacc, mybir
from concourse.bass_utils import run_bass_kernel_spmd

B, S, E, H = 4, 2048, 2048, 16
D = E // H            # 128 head dim
HL = 8                # heads per core
FL = HL * D           # 1024 local features
N_CORES = 8
ROPE_BASE = 10000.0
SCALE = float(D) ** -0.5
F32 = mybir.dt.float32
F32R = mybir.dt.float32r
BF16 = mybir.dt.bfloat16
Act = mybir.ActivationFunctionType

NE = E // 128         # 16 e-tiles
NT = S // 512         # 4 t-blocks of 512
NTT = S // 128        # 16 t-tiles of 128
NFO = FL // 128       # 8 f-tiles (= local heads)


def build_nc():
    nc = bacc.Bacc(
        "TRN2", target_bir_lowering=False, debug=False, num_devices=N_CORES
    )
    xT = nc.dram_tensor("xT", [E, S], BF16, kind="ExternalInput").ap()
    wq = nc.dram_tensor("wq", [E, FL], BF16, kind="ExternalInput").ap()
    wk = nc.dram_tensor("wk", [E, FL], BF16, kind="ExternalInput").ap()
    wv = nc.dram_tensor("wv", [E, FL], BF16, kind="ExternalInput").ap()
    wo = nc.dram_tensor("wo", [FL, E], BF16, kind="ExternalInput").ap()
    bq = nc.dram_tensor("bq", [128, NFO], F32, kind="ExternalInput").ap()
    bk = nc.dram_tensor("bk", [128, NFO], F32, kind="ExternalInput").ap()
    bv = nc.dram_tensor("bv", [1, FL], BF16, kind="ExternalInput").ap()
    cosT = nc.dram_tensor("cosT", [128, S], BF16, kind="ExternalInput").ap()
    sinST = nc.dram_tensor("sinST", [128, S], BF16, kind="ExternalInput").ap()
    tri = nc.dram_tensor("tri", [128, 128], BF16, kind="ExternalInput").ap()
    ones_col_d = nc.dram_tensor("ones_col", [128, 1], BF16, kind="ExternalInput").ap()
    ones_row_d = nc.dram_tensor("ones_row", [1, 128], F32R, kind="ExternalInput").ap()
    out = nc.dram_tensor("out", [S, E], F32, kind="ExternalOutput").ap()

    with tile.TileContext(nc) as tc, ExitStack() as top:
        dram = top.enter_context(tc.tile_pool(name="dram", bufs=1, space="DRAM"))
        v_d = dram.tile([S, FL], BF16)

        cpool = top.enter_context(tc.tile_pool(name="const", bufs=1))
        ones_col = cpool.tile([128, 1], BF16, tag="ones_col")
        nc.sync.dma_start(ones_col[:], ones_col_d[:])
        ones_row = cpool.tile([1, 128], F32R, tag="ones_row")
        nc.sync.dma_start(ones_row[:], ones_row_d[:])
        zb = cpool.tile([128, 1], F32, tag="zb")
        nc.gpsimd.memset(zb[:], 0.0)
        tri_sb = cpool.tile([128, 128], BF16, tag="tri")
        nc.sync.dma_start(tri_sb[:], tri[:])
        bq_sb = cpool.tile([128, NFO], F32, tag="bq")
        nc.sync.dma_start(bq_sb[:], bq[:])
        bk_sb = cpool.tile([128, NFO], F32, tag="bk")
        nc.sync.dma_start(bk_sb[:], bk[:])
        bv_sb = cpool.tile([1, FL], BF16, tag="bv")
        nc.sync.dma_start(bv_sb[:], bv[:])

        # Whole-kernel residents: Q/K (rope'd) and attention output.
        res = top.enter_context(tc.tile_pool(name="res", bufs=1))
        qT_res = res.tile([128, NFO, S], BF16, tag="qT")
        kT_res = res.tile([128, NFO, S], BF16, tag="kT")
        ao_res = res.tile([128, NFO, S], BF16, tag="ao")

        xT_r = xT.rearrange("(eo p) t -> p eo t", p=128)

        # ---- Phase 1: V/Q/K projections (V first: P2's first dependency
        # is v_d + qT/kT of head 0; V gets the longest DMA runway).
        with ExitStack() as ph:
            c1 = ph.enter_context(tc.tile_pool(name="c1", bufs=1))
            wp = ph.enter_context(tc.tile_pool(name="wqk", bufs=3))
            xp = ph.enter_context(tc.tile_pool(name="xs", bufs=2))
            ps = ph.enter_context(tc.tile_pool(name="ps1", bufs=6, space="PSUM"))
            st = ph.enter_context(tc.tile_pool(name="st1", bufs=3))

            def load_w_half(w_in, half):
                t = wp.tile([128, NE, 512], BF16, tag="w")
                nc.sync.dma_start(
                    t[:],
                    w_in.rearrange("(eo p) f -> p eo f", p=128)[
                        :, :, half * 512 : (half + 1) * 512
                    ],
                )
                return t

            wv_half = [load_w_half(wv, 0)]
            x_first = xp.tile([128, NE, 512], BF16, tag="x")
            nc.sync.dma_start(x_first[:], xT_r[:, :, 0:512])
            wv_half.append(load_w_half(wv, 1))
            for tb in range(NT):
                if tb == 0:
                    x_sb = x_first
                else:
                    x_sb = xp.tile([128, NE, 512], BF16, tag="x")
                    nc.sync.dma_start(
                        x_sb[:], xT_r[:, :, tb * 512 : (tb + 1) * 512]
                    )
                for ttl in range(4):
                    tt = tb * 4 + ttl
                    for fo2 in range(2):
                        acc = ps.tile([128, 512], F32)
                        for eo in range(NE):
                            nc.tensor.matmul(
                                acc[:],
                                x_sb[:, eo, ttl * 128 : (ttl + 1) * 128],
                                wv_half[fo2][:, eo, :],
                                start=(eo == 0),
                                stop=(eo == NE - 1),
                            )
                        vt = st.tile([128, 512], BF16, tag="raw")
                        nc.scalar.copy(vt[:], acc[:])
                        nc.sync.dma_start(
                            v_d[tt * 128 : (tt + 1) * 128,
                                fo2 * 512 : (fo2 + 1) * 512],
                            vt[:],
                        )

            cos_sb = c1.tile([128, S], BF16, tag="cos")
            nc.sync.dma_start(cos_sb[:], cosT[:])
            sin_sb = c1.tile([128, S], BF16, tag="sin")
            nc.sync.dma_start(sin_sb[:], sinST[:])

            # Q and K passes: bias + RoPE (evens|odds layout: pair swap is a
            # half-partition swap), written straight into resident SBUF.
            for w_in, b_sb, dst in ((wq, bq_sb, qT_res), (wk, bk_sb, kT_res)):
                w_half = [load_w_half(w_in, 0), load_w_half(w_in, 1)]
                for tb in range(NT):
                    x_sb = xp.tile([128, NE, 512], BF16, tag="x")
                    nc.sync.dma_start(
                        x_sb[:], xT_r[:, :, tb * 512 : (tb + 1) * 512]
                    )
                    ts = slice(tb * 512, (tb + 1) * 512)
                    for fo in range(NFO):
                        w_sb = w_half[fo // 4]
                        fl = (fo % 4) * 128
                        acc = ps.tile([128, 512], F32)
                        for eo in range(NE):
                            nc.tensor.matmul(
                                acc[:],
                                w_sb[:, eo, fl : fl + 128],
                                x_sb[:, eo, :],
                                start=(eo == 0),
                                stop=(eo == NE - 1),
                            )
                        raw = st.tile([128, 512], BF16, tag="raw")
                        nc.scalar.activation(
                            raw[:], acc[:], Act.Identity,
                            bias=b_sb[:, fo : fo + 1],
                        )
                        # Half-partition swap via SBUF->SBUF DMA (DVE lanes
                        # cannot cross partitions); issued on the idle
                        # gpsimd queue.
                        raws = st.tile([128, 512], BF16, tag="raws")
                        nc.gpsimd.dma_start(raws[0:64, :], raw[64:128, :])
                        nc.gpsimd.dma_start(raws[64:128, :], raw[0:64, :])
                        u = st.tile([128, 512], BF16, tag="u")
                        nc.vector.tensor_mul(u[:], raw[:], cos_sb[:, ts])
                        w2 = st.tile([128, 512], BF16, tag="w2")
                        nc.vector.tensor_mul(w2[:], raws[:], sin_sb[:, ts])
                        nc.vector.tensor_add(dst[:, fo, ts], u[:], w2[:])

        # Wo prefetch pool opened after P1 frees its SBUF; DMAs overlap P2.
        wp3 = top.enter_context(tc.tile_pool(name="wo", bufs=1))
        wo_sb = wp3.tile([128, NFO, E], BF16)
        wo_r = wo.rearrange("(fo p) e -> p fo e", p=128)
        for fo in range(NFO):
            nc.sync.dma_start(wo_sb[:, fo, :], wo_r[:, fo, :])

        # ---- Phase 2: attention per head -> ao_res (resident, bf16)
        with ExitStack() as ph:
            hp = ph.enter_context(tc.tile_pool(name="heads", bufs=2))
            ep = ph.enter_context(tc.tile_pool(name="expS", bufs=2))
            psS = ph.enter_context(tc.tile_pool(name="psS", bufs=2, space="PSUM"))
            psSum = ph.enter_context(
                tc.tile_pool(name="psSum", bufs=1, space="PSUM")
            )
            psO = ph.enter_context(tc.tile_pool(name="psO", bufs=2, space="PSUM"))
            psB = ph.enter_context(tc.tile_pool(name="psB", bufs=1, space="PSUM"))
            sm = ph.enter_context(tc.tile_pool(name="sm2", bufs=2))
            st = ph.enter_context(tc.tile_pool(name="st2", bufs=2))

            pend = None  # deferred normalize: (pso, recip, h, b)

            def flush_normalize():
                nonlocal pend
                if pend is None:
                    return
                pso, recip, h, b = pend
                psb = psB.tile([128, 512], F32)
                nc.tensor.matmul(
                    psb[:], ones_row[:], recip[:],
                    start=True, stop=True,
                )
                bcast = st.tile([128, 512], F32, tag="bcast")
                nc.vector.tensor_copy(bcast[:], psb[:])
                nc.vector.tensor_mul(
                    ao_res[:, h, b * 512 : (b + 1) * 512], pso[:], bcast[:]
                )
                pend = None

            for h in range(HL):
                v_h = hp.tile([128, NTT, 128], BF16, tag="vh")
                nc.sync.dma_start(
                    v_h[:],
                    v_d[:, h * 128 : (h + 1) * 128].rearrange(
                        "(j p) d -> p j d", p=128
                    ),
                )
                eS = ep.tile([128, NTT, 512], BF16)
                for b in range(NT):
                    nk = 4 * b + 4
                    # logits + exp (exp batched over 2-bank PSUM pairs for
                    # the full-width tiles; diagonal partials done singly)
                    j = 0
                    while j < nk:
                        m = j - 4 * b
                        if m < 0 and j + 1 < 4 * b:  # two full tiles
                            ls = psS.tile([128, 2, 512], F32)
                            for jj in range(2):
                                nc.tensor.matmul(
                                    ls[:, jj, :],
                                    kT_res[:, h, (j + jj) * 128 : (j + jj + 1) * 128],
                                    qT_res[:, h, b * 512 : (b + 1) * 512],
                                    start=True, stop=True,
                                )
                            nc.scalar.activation(
                                eS[:, j : j + 2, :], ls[:, :, :], Act.Exp,
                                bias=zb[:, 0:1], scale=SCALE,
                            )
                            j += 2
                        else:
                            q0 = 128 * max(m, 0)
                            ls = psS.tile([128, 2, 512], F32)
                            nc.tensor.matmul(
                                ls[:, 0, q0:512],
                                kT_res[:, h, j * 128 : (j + 1) * 128],
                                qT_res[:, h, b * 512 + q0 : (b + 1) * 512],
                                start=True, stop=True,
                            )
                            nc.scalar.activation(
                                eS[:, j, q0:512], ls[:, 0, q0:512], Act.Exp,
                                bias=zb[:, 0:1], scale=SCALE,
                            )
                            if m >= 0:  # true-diagonal 128x128: triangle mask
                                nc.vector.tensor_mul(
                                    eS[:, j, q0 : q0 + 128],
                                    eS[:, j, q0 : q0 + 128],
                                    tri_sb[:],
                                )
                            j += 1
                    # row sums via ones-vector matmuls
                    ssum = psSum.tile([1, 512], F32)
                    for j in range(nk):
                        q0 = 128 * max(j - 4 * b, 0)
                        nc.tensor.matmul(
                            ssum[:, q0:512], ones_col[:], eS[:, j, q0:512],
                            start=(j == 0), stop=(j == nk - 1),
                        )
                    sums_bf = sm.tile([1, 512], BF16, tag="sums")
                    nc.scalar.copy(sums_bf[:], ssum[:])
                    recip_f = sm.tile([1, 512], F32, tag="recipf")
                    nc.vector.reciprocal_approx_fast(recip_f[:], ssum[:])
                    recip = sm.tile([1, 512], F32R, tag="recip")
                    with nc.allow_low_precision(reason="f32r matmul operand"):
                        nc.vector.tensor_copy(recip[:], recip_f[:])
                    # PV (+ rank-1 bv*sums fold)
                    pso = psO.tile([128, 512], F32)
                    for j in range(nk):
                        q0 = 128 * max(j - 4 * b, 0)
                        nc.tensor.matmul(
                            pso[:, q0:512], v_h[:, j, :], eS[:, j, q0:512],
                            start=(j == 0), stop=False,
                        )
                    nc.tensor.matmul(
                        pso[:],
                        bv_sb[0:1, h * 128 : (h + 1) * 128],
                        sums_bf[:],
                        start=False, stop=True,
                    )
                    flush_normalize()
                    pend = (pso, recip, h, b)
            flush_normalize()

        # ---- Phase 3: out proj -> out [t, e] (bo added on host)
        with ExitStack() as ph:
            ps = ph.enter_context(tc.tile_pool(name="ps3", bufs=4, space="PSUM"))
            st = ph.enter_context(tc.tile_pool(name="st3", bufs=3))
            for tt in range(NTT):
                for eb in range(NT):
                    acc = ps.tile([128, 512], F32)
                    for fo in range(NFO):
                        nc.tensor.matmul(
                            acc[:],
                            ao_res[:, fo, tt * 128 : (tt + 1) * 128],
                            wo_sb[:, fo, eb * 512 : (eb + 1) * 512],
                            start=(fo == 0),
                            stop=(fo == NFO - 1),
                        )
                    osb = st.tile([128, 512], F32)
                    nc.scalar.copy(osb[:], acc[:])
                    nc.sync.dma_start(
                        out[tt * 128 : (tt + 1) * 128,
                            eb * 512 : (eb + 1) * 512],
                        osb[:],
                    )

    nc.compile()
    return nc


def _host_inputs(x, Wq, bq, Wk, bk, Wv, bv, Wo, bo):
    BF = ml_dtypes.bfloat16
    # Per-head feature permutation: evens then odds (RoPE half-swap layout).
    perm1 = np.concatenate([np.arange(0, D, 2), np.arange(1, D, 2)])
    # RoPE tables in [d, t] layout for the permuted basis.
    i = np.arange(0, D, 2, dtype=np.float64)
    invf = ROPE_BASE ** (-i / D)                      # (64,)
    pos = np.arange(S, dtype=np.float64)
    ang = pos[None, :] * invf[:, None]                # (64, S)
    cosT = np.concatenate([np.cos(ang), np.cos(ang)], 0).astype(np.float32)
    # sin table signs match the half-SWAPPED operand (raws): row i<64 holds
    # -sin_i (multiplies o_i), row 64+i holds +sin_i (multiplies e_i).
    sinST = np.concatenate([-np.sin(ang), np.sin(ang)], 0).astype(np.float32)
    # Triangle mask for the true-diagonal 128x128 tile: [k, q], q >= k.
    ki = np.arange(128)[:, None]
    qi = np.arange(128)[None, :]
    tri = (qi >= ki).astype(np.float32)

    in_maps = []
    for c in range(N_CORES):
        bi, g = c % B, c // B
        sl = slice(g * FL, (g + 1) * FL)
        fperm = (np.arange(FL).reshape(NFO, D)[:, perm1]).reshape(FL)
        wq_l = Wq[sl, :][fperm, :]
        wk_l = Wk[sl, :][fperm, :]
        bq_l = bq[sl][fperm]
        bk_l = bk[sl][fperm]
        in_maps.append({
            "xT": np.ascontiguousarray(x[bi].T).astype(BF),
            "wq": np.ascontiguousarray(wq_l.T).astype(BF),
            "wk": np.ascontiguousarray(wk_l.T).astype(BF),
            "wv": np.ascontiguousarray(Wv[sl, :].T).astype(BF),
            "wo": np.ascontiguousarray(Wo[:, sl].T).astype(BF),
            "bq": np.ascontiguousarray(bq_l.reshape(NFO, 128).T).astype(np.float32),
            "bk": np.ascontiguousarray(bk_l.reshape(NFO, 128).T).astype(np.float32),
            "bv": bv[sl].reshape(1, FL).astype(BF),
            "cosT": cosT.astype(BF),
            "sinST": sinST.astype(BF),
            "tri": tri.astype(BF),
            "ones_col": np.ones((128, 1), np.float32).astype(BF),
            "ones_row": np.ones((1, 128), np.float32),
        })
    return in_maps


_NC_CACHE = {}


def _install_ntff_hook():
    """Recreate the missing antenv.axon_hooks module so trace=True works."""
    import sys, types, ctypes, contextlib

    if "antenv.axon_hooks" in sys.modules:
        return
    so_path = "/opt/axon/libaxon_pjrt.so"
    lib = ctypes.CDLL(so_path)
    lib.axon_start_nrt_profile.argtypes = [
        ctypes.POINTER(ctypes.c_int64), ctypes.c_size_t,
    ]
    lib.axon_start_nrt_profile.restype = ctypes.c_int64
    lib.axon_stop_nrt_profile.argtypes = [ctypes.c_char_p]
    lib.axon_stop_nrt_profile.restype = ctypes.c_int64

    @contextlib.contextmanager
    def _hook(output_dir, device_ids):
        import jax
        jax.devices()
        if device_ids:
            ids = (ctypes.c_int64 * len(device_ids))(*device_ids)
            rc = lib.axon_start_nrt_profile(ids, len(device_ids))
        else:
            rc = lib.axon_start_nrt_profile(None, 0)
        if rc != 0:
            raise RuntimeError(f"axon_start_nrt_profile rc={rc}")
        try:
            yield
        finally:
            n = lib.axon_stop_nrt_profile(str(output_dir).encode())
            print(f"ntff profile: {n} file(s) -> {output_dir}", flush=True)

    mod = types.ModuleType("antenv.axon_hooks")
    mod.get_axon_ntff_profile_hook = lambda: _hook
    mod.set_axon_ntff_profile_hook = lambda h: None
    sys.modules["antenv.axon_hooks"] = mod
    import concourse.bass_utils as bu
    bu.upload_artifacts = lambda tmpdir: f"file://{tmpdir}"


def run(x, Wq, bq, Wk, bk, Wv, bv, Wo, bo, trace=False):
    if trace:
        try:
            _install_ntff_hook()
        except Exception as e:
            print(f"ntff hook install failed ({e}); tracing may degrade")
    if "nc" not in _NC_CACHE:
        _NC_CACHE["nc"] = build_nc()
    nc = _NC_CACHE["nc"]
    in_maps = _host_inputs(x, Wq, bq, Wk, bk, Wv, bv, Wo, bo)
    res = run_bass_kernel_spmd(
        nc, in_maps, core_ids=list(range(N_CORES)), trace=trace
    )
    outs = [res.results[c]["out"] for c in range(N_CORES)]
    full = np.empty((B, S, E), np.float32)
    for bi in range(B):
        full[bi] = outs[bi] + outs[bi + B] + bo[None, :]
    return full, res


def kernel(**inputs):
    full, _ = run(**inputs)
    return full
